# revision 6
# baseline (speedup 1.0000x reference)
"""Viterbi CRF decode (nn_CRF) on TRN2 NeuronCores via Bass/Tile.

The reference returns only (scores[-1], paths[-1]) — the decode of batch
element 63 — so the kernel runs Viterbi on that single sequence, bit-exactly
reproducing the reference's fp32 arithmetic:

  s_0 = [-1e4 ... 0 @ START ... -1e4]
  scores_t[n, p] = transitions[n, p] + s_t[p]          (one fp32 add)
  vit_t[n] = max_p scores_t[n, p]                      (exact max)
  bptr_t[n] = argmax_p scores_t[n, p]                  (first max index)
  s_{t+1}[n] = vit_t[n] + feats[t, n]                  (one fp32 add)
  (steps with t >= length leave the reference state frozen; equivalently we
   run unmasked and read s_L for the terminal, ignoring bptrs at t >= L)

Phase A (forward chain, 1 core). State vectors use "column layout"
([128 partitions, 4 cols]; element n -> (n % 128, n // 128)). Per step the
PSUM banks accumulate scores entirely on TensorE via transpose-mode matmuls
(pure fp32 data movement, so bit-exact; PSUM accumulation provides the single
fp32 add per element):
  bank_i[:, 128q:128(q+1)]  = transpose(T4T block(i,q))      # transitions
  bank_i[:, 128q:128(q+1)] += transpose(bcast(s column q))   # s_t broadcast
The s broadcast feeds the transpose a free-step-0 (to_broadcast) view of the
history column: a row-constant stationary tile whose transpose is the
partition-broadcast of s. VectorE then row-max-reduces each bank into the vit
history and adds feats to extend the s history. The argmax is NOT computed
here — it is recomputed in parallel in phase B.

Phase B (8 cores, data-parallel over the 1024 steps, 128 each): rebuild the
score banks the same way from the phase-A history, ScalarE copies PSUM->SBUF,
and the DVE max_index instruction extracts argmax (first-index semantics,
identical to jnp.argmax) using the exact vit value from phase A as the key.

Host: terminal scoring from s_L + transitions[STOP], numpy backtrace,
padding with -1 beyond the sequence length (as the reference does).
"""

import os
from contextlib import ExitStack

import numpy as np

import concourse.bacc as bacc
import concourse.mybir as mybir
import concourse.tile as tile
from concourse.bass_utils import run_bass_kernel_spmd

F32 = mybir.dt.float32
U32 = mybir.dt.uint32

B = 64
T = 1024
K = 512
KT = 4  # K // 128
SEQ = B - 1
START_TAG = 510
STOP_TAG = 511
NEG_INF = -10000.0
N_CORES = 8
S = T // N_CORES  # steps per core in phase B

PROFILE = bool(int(os.environ.get("VITERBI_PROFILE", "0")))
LAST_EXEC_NS = {}

_CACHE = {}


def _build_phase_a():
    nc = bacc.Bacc("TRN2", target_bir_lowering=False, debug=False)
    t4t_d = nc.dram_tensor("t4t", [128, KT * K], F32, kind="ExternalInput")
    feat_d = nc.dram_tensor("feat4", [128, KT * T], F32, kind="ExternalInput")
    vf0_d = nc.dram_tensor("vf0", [128, KT], F32, kind="ExternalInput")
    id_d = nc.dram_tensor("ident", [128, 128], F32, kind="ExternalInput")
    vfh_d = nc.dram_tensor("vf_hist", [128, KT * (T + 1)], F32, kind="ExternalOutput")
    vith_d = nc.dram_tensor("vit_hist", [128, KT * T], F32, kind="ExternalOutput")

    with tile.TileContext(nc) as tc:
        with ExitStack() as ctx:
            persist = ctx.enter_context(tc.tile_pool(name="persist", bufs=1))
            ps_pool = ctx.enter_context(
                tc.tile_pool(name="pspool", bufs=2, space="PSUM")
            )

            t4t_sb = persist.tile([128, KT * K], F32, name="t4t_sb")
            feat_sb = persist.tile([128, KT * T], F32, name="feat_sb")
            id_sb = persist.tile([128, 128], F32, name="id_sb")
            vfh = persist.tile([128, KT * (T + 1)], F32, name="vfh")
            vith = persist.tile([128, KT * T], F32, name="vith")

            nc.sync.dma_start(t4t_sb, t4t_d.ap())
            nc.sync.dma_start(feat_sb, feat_d.ap())
            nc.sync.dma_start(id_sb, id_d.ap())
            nc.sync.dma_start(vfh[:, 0:KT], vf0_d.ap())

            for t in range(T):
                banks = [
                    ps_pool.tile([128, K], F32, name=f"bank{i}", tag=f"bank{i}")
                    for i in range(KT)
                ]
                for i in range(KT):
                    for q in range(KT):
                        nc.tensor.matmul(
                            banks[i][:, 128 * q : 128 * (q + 1)],
                            t4t_sb[:, 128 * (KT * i + q) : 128 * (KT * i + q + 1)],
                            id_sb,
                            is_transpose=True,
                            start=(q == 0),  # zeroes the whole bank
                            stop=False,
                            skip_group_check=True,
                        )
                    for q in range(KT):
                        # s_t broadcast: transpose of the row-constant view of
                        # the history column (free-step-0 stationary AP)
                        nc.tensor.matmul(
                            banks[i][:, 128 * q : 128 * (q + 1)],
                            vfh[:, KT * t + q : KT * t + q + 1].to_broadcast(
                                [128, 128]
                            ),
                            id_sb,
                            is_transpose=True,
                            start=False,
                            stop=(q == KT - 1),
                            skip_group_check=True,
                        )
                for i in range(KT):
                    nc.vector.tensor_reduce(
                        out=vith[:, KT * t + i : KT * t + i + 1],
                        in_=banks[i],
                        axis=mybir.AxisListType.X,
                        op=mybir.AluOpType.max,
                    )
                    nc.vector.tensor_add(
                        vfh[:, KT * (t + 1) + i : KT * (t + 1) + i + 1],
                        vith[:, KT * t + i : KT * t + i + 1],
                        feat_sb[:, KT * t + i : KT * t + i + 1],
                    )

            nc.sync.dma_start(vfh_d.ap(), vfh)
            nc.sync.dma_start(vith_d.ap(), vith)

    nc.compile()
    return nc


def _build_phase_b():
    nc = bacc.Bacc("TRN2", target_bir_lowering=False, debug=False)
    t4t_d = nc.dram_tensor("t4t", [128, KT * K], F32, kind="ExternalInput")
    vfs_d = nc.dram_tensor("vf_slice", [128, KT * S], F32, kind="ExternalInput")
    vits_d = nc.dram_tensor("vit_slice", [128, KT * S], F32, kind="ExternalInput")
    id_d = nc.dram_tensor("ident", [128, 128], F32, kind="ExternalInput")
    bidx_d = nc.dram_tensor("bidx", [128, 8 * KT * S], U32, kind="ExternalOutput")

    with tile.TileContext(nc) as tc:
        with ExitStack() as ctx:
            persist = ctx.enter_context(tc.tile_pool(name="persist", bufs=1))
            sc_pool = ctx.enter_context(tc.tile_pool(name="scpool", bufs=4))
            ps_pool = ctx.enter_context(
                tc.tile_pool(name="pspool", bufs=2, space="PSUM")
            )

            t4t_sb = persist.tile([128, KT * K], F32, name="t4t_sb")
            vfs_sb = persist.tile([128, KT * S], F32, name="vfs_sb")
            vits_sb = persist.tile([128, KT * S], F32, name="vits_sb")
            id_sb = persist.tile([128, 128], F32, name="id_sb")
            bidx = persist.tile([128, 8 * KT * S], U32, name="bidx")

            nc.sync.dma_start(t4t_sb, t4t_d.ap())
            nc.sync.dma_start(vfs_sb, vfs_d.ap())
            nc.sync.dma_start(vits_sb, vits_d.ap())
            nc.sync.dma_start(id_sb, id_d.ap())

            for s in range(S):
                banks = [
                    ps_pool.tile([128, K], F32, name=f"bank{i}", tag=f"bank{i}")
                    for i in range(KT)
                ]
                for i in range(KT):
                    for q in range(KT):
                        nc.tensor.matmul(
                            banks[i][:, 128 * q : 128 * (q + 1)],
                            t4t_sb[:, 128 * (KT * i + q) : 128 * (KT * i + q + 1)],
                            id_sb,
                            is_transpose=True,
                            start=(q == 0),
                            stop=False,
                            skip_group_check=True,
                        )
                    for q in range(KT):
                        nc.tensor.matmul(
                            banks[i][:, 128 * q : 128 * (q + 1)],
                            vfs_sb[:, KT * s + q : KT * s + q + 1].to_broadcast(
                                [128, 128]
                            ),
                            id_sb,
                            is_transpose=True,
                            start=False,
                            stop=(q == KT - 1),
                            skip_group_check=True,
                        )
                scores = sc_pool.tile([128, KT * K], F32, name="scores", tag="scores")
                for i in range(KT):
                    nc.scalar.copy(scores[:, K * i : K * (i + 1)], banks[i])
                for i in range(KT):
                    nc.vector.max_index(
                        out=bidx[:, 32 * s + 8 * i : 32 * s + 8 * (i + 1)],
                        in_max=vits_sb[:, KT * s + i : KT * s + i + 1].to_broadcast(
                            [128, 8]
                        ),
                        in_values=scores[:, K * i : K * (i + 1)],
                    )

            nc.sync.dma_start(bidx_d.ap(), bidx)

    nc.compile()
    return nc


def _get_programs():
    if "a" not in _CACHE:
        _CACHE["a"] = _build_phase_a()
    if "b" not in _CACHE:
        _CACHE["b"] = _build_phase_b()
    return _CACHE["a"], _CACHE["b"]


def kernel(feats: np.ndarray, transitions: np.ndarray, lengths: np.ndarray):
    feats = np.asarray(feats)
    transitions = np.asarray(transitions, dtype=np.float32)
    lengths = np.asarray(lengths)
    assert feats.shape == (B, T, K) and transitions.shape == (K, K)

    seq = np.ascontiguousarray(feats[SEQ], dtype=np.float32)  # [T, K]
    length = int(lengths[SEQ])

    # host-side layout prep (cheap)
    # T4T[j, 128*(4*i+q) + m] = transitions[128*i + m, 128*q + j]
    t4t = np.ascontiguousarray(
        transitions.reshape(KT, 128, KT, 128)
        .transpose(3, 0, 2, 1)
        .reshape(128, KT * KT * 128)
    )
    f4 = np.ascontiguousarray(
        seq.reshape(T, KT, 128).transpose(2, 0, 1).reshape(128, T * KT)
    )
    s0 = np.full(K, NEG_INF, dtype=np.float32)
    s0[START_TAG] = 0.0
    vf0 = np.ascontiguousarray(s0.reshape(KT, 128).T)
    ident = np.eye(128, dtype=np.float32)

    nc_a, nc_b = _get_programs()

    if PROFILE and "model_a" not in LAST_EXEC_NS:
        # No NTFF profiling under axon in this deployment; report the
        # instruction-cost-model timeline (same model the Tile scheduler uses).
        from concourse.timeline_sim import TimelineSim

        LAST_EXEC_NS["model_a"] = int(TimelineSim(nc_a).simulate())
        LAST_EXEC_NS["model_b"] = int(TimelineSim(nc_b).simulate())

    res_a = run_bass_kernel_spmd(
        nc_a,
        [{"t4t": t4t, "feat4": f4, "vf0": vf0, "ident": ident}],
        core_ids=[0],
    )
    vfh = res_a.results[0]["vf_hist"]  # [128, 4*(T+1)]
    vith = res_a.results[0]["vit_hist"]  # [128, 4*T]

    in_maps = []
    for c in range(N_CORES):
        in_maps.append(
            {
                "t4t": t4t,
                "vf_slice": np.ascontiguousarray(vfh[:, KT * S * c : KT * S * (c + 1)]),
                "vit_slice": np.ascontiguousarray(
                    vith[:, KT * S * c : KT * S * (c + 1)]
                ),
                "ident": ident,
            }
        )
    res_b = run_bass_kernel_spmd(nc_b, in_maps, core_ids=list(range(N_CORES)))

    # bptrs[t, n] = first argmax over prev
    bptrs = np.empty((T, K), dtype=np.int64)
    for c in range(N_CORES):
        bidx = res_b.results[c]["bidx"]  # [128, 32*S]
        view = bidx.reshape(128, S, KT, 8)[:, :, :, 0]  # [128, S, KT]
        bptrs[S * c : S * (c + 1)] = view.transpose(1, 2, 0).reshape(S, K)

    # s_L (state after `length` steps; the reference freezes beyond that)
    s_l = vfh[:, KT * length : KT * (length + 1)].T.reshape(K)
    terminal = s_l + transitions[STOP_TAG]
    best = int(np.argmax(terminal))
    score = np.float32(terminal[best])

    int_dtype = lengths.dtype if lengths.dtype in (np.int32, np.int64) else np.int64
    path = np.full(T, -1, dtype=int_dtype)
    tag = best
    for t in range(T - 1, -1, -1):
        if t < length:
            path[t] = tag
            tag = int(bptrs[t, tag])
    return score, path


# revision 7
# speedup vs baseline: 1.0116x; 1.0116x over previous
"""Viterbi CRF decode (nn_CRF) on TRN2 NeuronCores via Bass/Tile.

The reference returns only (scores[-1], paths[-1]) — the decode of batch
element 63 — so the kernel runs Viterbi on that single sequence, bit-exactly
reproducing the reference's fp32 arithmetic:

  s_0 = [-1e4 ... 0 @ START ... -1e4]
  scores_t[n, p] = transitions[n, p] + s_t[p]          (one fp32 add)
  vit_t[n] = max_p scores_t[n, p]                      (exact max)
  bptr_t[n] = argmax_p scores_t[n, p]                  (first max index)
  s_{t+1}[n] = vit_t[n] + feats[t, n]                  (one fp32 add)
  (steps with t >= length leave the reference state frozen; equivalently we
   run unmasked and read s_L for the terminal, ignoring bptrs at t >= L)

Phase A (forward chain, 1 core). State vectors use "column layout"
([128 partitions, 4 cols]; element n -> (n % 128, n // 128)). Per step the
PSUM banks accumulate scores entirely on TensorE via transpose-mode matmuls
(pure fp32 data movement, so bit-exact; PSUM accumulation provides the single
fp32 add per element):
  bank_i[:, 128q:128(q+1)]  = transpose(T4T block(i,q))      # transitions
  bank_i[:, 128q:128(q+1)] += transpose(bcast(s column q))   # s_t broadcast
The s broadcast feeds the transpose a free-step-0 (to_broadcast) view of the
history column: a row-constant stationary tile whose transpose is the
partition-broadcast of s. VectorE then row-max-reduces each bank into the vit
history and adds feats to extend the s history. The argmax is NOT computed
here — it is recomputed in parallel in phase B.

Phase B (8 cores, data-parallel over the 1024 steps, 128 each): rebuild the
score banks the same way from the phase-A history, ScalarE copies PSUM->SBUF,
and the DVE max_index instruction extracts argmax (first-index semantics,
identical to jnp.argmax) using the exact vit value from phase A as the key.

Host: terminal scoring from s_L + transitions[STOP], numpy backtrace,
padding with -1 beyond the sequence length (as the reference does).
"""

import os
from contextlib import ExitStack

import numpy as np

import concourse.bacc as bacc
import concourse.mybir as mybir
import concourse.tile as tile
from concourse.bass_utils import run_bass_kernel_spmd

F32 = mybir.dt.float32
U32 = mybir.dt.uint32

B = 64
T = 1024
K = 512
KT = 4  # K // 128
SEQ = B - 1
START_TAG = 510
STOP_TAG = 511
NEG_INF = -10000.0
N_CORES = 8
S = T // N_CORES  # steps per core in phase B

PROFILE = bool(int(os.environ.get("VITERBI_PROFILE", "0")))
LAST_EXEC_NS = {}

_CACHE = {}


def _build_phase_a():
    nc = bacc.Bacc("TRN2", target_bir_lowering=False, debug=False)
    t4t_d = nc.dram_tensor("t4t", [128, KT * K], F32, kind="ExternalInput")
    feat_d = nc.dram_tensor("feat4", [128, KT * T], F32, kind="ExternalInput")
    vf0_d = nc.dram_tensor("vf0", [128, KT], F32, kind="ExternalInput")
    id_d = nc.dram_tensor("ident", [128, 128], F32, kind="ExternalInput")
    vfh_d = nc.dram_tensor("vf_hist", [128, KT * (T + 1)], F32, kind="ExternalOutput")
    vith_d = nc.dram_tensor("vit_hist", [128, KT * T], F32, kind="ExternalOutput")

    with tile.TileContext(nc) as tc:
        with ExitStack() as ctx:
            persist = ctx.enter_context(tc.tile_pool(name="persist", bufs=1))
            ps_pool = ctx.enter_context(
                tc.tile_pool(name="pspool", bufs=2, space="PSUM")
            )

            t4t_sb = persist.tile([128, KT * K], F32, name="t4t_sb")
            feat_sb = persist.tile([128, KT * T], F32, name="feat_sb")
            id_sb = persist.tile([128, 128], F32, name="id_sb")
            vfh = persist.tile([128, KT * (T + 1)], F32, name="vfh")
            vith = persist.tile([128, KT * T], F32, name="vith")

            nc.sync.dma_start(t4t_sb, t4t_d.ap())
            nc.sync.dma_start(feat_sb, feat_d.ap())
            nc.sync.dma_start(id_sb, id_d.ap())
            nc.sync.dma_start(vfh[:, 0:KT], vf0_d.ap())

            for t in range(T):
                banks = [
                    ps_pool.tile([128, K], F32, name=f"bank{i}", tag=f"bank{i}")
                    for i in range(KT)
                ]
                for i in range(KT):
                    for q in range(KT):
                        nc.tensor.matmul(
                            banks[i][:, 128 * q : 128 * (q + 1)],
                            t4t_sb[:, 128 * (KT * i + q) : 128 * (KT * i + q + 1)],
                            id_sb,
                            is_transpose=True,
                            start=(q == 0),  # zeroes the whole bank
                            stop=False,
                            skip_group_check=True,
                        )
                    for q in range(KT):
                        # s_t broadcast: transpose of the row-constant view of
                        # the history column (free-step-0 stationary AP)
                        nc.tensor.matmul(
                            banks[i][:, 128 * q : 128 * (q + 1)],
                            vfh[:, KT * t + q : KT * t + q + 1].to_broadcast(
                                [128, 128]
                            ),
                            id_sb,
                            is_transpose=True,
                            start=False,
                            stop=(q == KT - 1),
                            skip_group_check=True,
                        )
                for i in range(KT):
                    nc.vector.tensor_reduce(
                        out=vith[:, KT * t + i : KT * t + i + 1],
                        in_=banks[i],
                        axis=mybir.AxisListType.X,
                        op=mybir.AluOpType.max,
                    )
                    nc.vector.tensor_add(
                        vfh[:, KT * (t + 1) + i : KT * (t + 1) + i + 1],
                        vith[:, KT * t + i : KT * t + i + 1],
                        feat_sb[:, KT * t + i : KT * t + i + 1],
                    )

            nc.sync.dma_start(vfh_d.ap(), vfh)
            nc.sync.dma_start(vith_d.ap(), vith)

    nc.compile()
    return nc


def _build_phase_b():
    nc = bacc.Bacc("TRN2", target_bir_lowering=False, debug=False)
    t4t_d = nc.dram_tensor("t4t", [128, KT * K], F32, kind="ExternalInput")
    t4_d = nc.dram_tensor("t4", [128, KT * K], F32, kind="ExternalInput")
    vfs_d = nc.dram_tensor("vf_slice", [128, KT * S], F32, kind="ExternalInput")
    vits_d = nc.dram_tensor("vit_slice", [128, KT * S], F32, kind="ExternalInput")
    id_d = nc.dram_tensor("ident", [128, 128], F32, kind="ExternalInput")
    bidx_d = nc.dram_tensor("bidx", [128, 8 * KT * S], U32, kind="ExternalOutput")

    with tile.TileContext(nc) as tc:
        with ExitStack() as ctx:
            persist = ctx.enter_context(tc.tile_pool(name="persist", bufs=1))
            sc_pool = ctx.enter_context(tc.tile_pool(name="scpool", bufs=4))
            ps_pool = ctx.enter_context(
                tc.tile_pool(name="pspool", bufs=2, space="PSUM")
            )

            t4t_sb = persist.tile([128, KT * K], F32, name="t4t_sb")
            t4_sb = persist.tile([128, KT * K], F32, name="t4_sb")
            vfs_sb = persist.tile([128, KT * S], F32, name="vfs_sb")
            vits_sb = persist.tile([128, KT * S], F32, name="vits_sb")
            id_sb = persist.tile([128, 128], F32, name="id_sb")
            bidx = persist.tile([128, 8 * KT * S], U32, name="bidx")

            nc.sync.dma_start(t4t_sb, t4t_d.ap())
            nc.sync.dma_start(t4_sb, t4_d.ap())
            nc.sync.dma_start(vfs_sb, vfs_d.ap())
            nc.sync.dma_start(vits_sb, vits_d.ap())
            nc.sync.dma_start(id_sb, id_d.ap())

            NPE = KT - 1  # tiles on the PE transpose path; last tile on DVE
            for s in range(S):
                scores = sc_pool.tile([128, KT * K], F32, name="scores", tag="scores")
                banks = [
                    ps_pool.tile([128, K], F32, name=f"bank{i}", tag=f"bank{i}")
                    for i in range(NPE)
                ]
                for i in range(NPE):
                    for q in range(KT):
                        nc.tensor.matmul(
                            banks[i][:, 128 * q : 128 * (q + 1)],
                            t4t_sb[:, 128 * (KT * i + q) : 128 * (KT * i + q + 1)],
                            id_sb,
                            is_transpose=True,
                            start=(q == 0),
                            stop=False,
                            skip_group_check=True,
                        )
                    for q in range(KT):
                        nc.tensor.matmul(
                            banks[i][:, 128 * q : 128 * (q + 1)],
                            vfs_sb[:, KT * s + q : KT * s + q + 1].to_broadcast(
                                [128, 128]
                            ),
                            id_sb,
                            is_transpose=True,
                            start=False,
                            stop=(q == KT - 1),
                            skip_group_check=True,
                        )
                for i in range(NPE):
                    nc.scalar.copy(scores[:, K * i : K * (i + 1)], banks[i])
                # DVE-path tile: s broadcast into an fvb bank, one TT add
                fvb = ps_pool.tile([128, K], F32, name="fvb", tag="fvb")
                for q in range(KT):
                    nc.tensor.matmul(
                        fvb[:, 128 * q : 128 * (q + 1)],
                        vfs_sb[:, KT * s + q : KT * s + q + 1].to_broadcast([128, 128]),
                        id_sb,
                        is_transpose=True,
                        start=(q == 0),
                        stop=(q == KT - 1),
                        skip_group_check=True,
                    )
                for i in range(NPE, KT):
                    nc.vector.tensor_add(
                        scores[:, K * i : K * (i + 1)], t4_sb[:, K * i : K * (i + 1)], fvb
                    )
                for i in range(KT):
                    nc.vector.max_index(
                        out=bidx[:, 32 * s + 8 * i : 32 * s + 8 * (i + 1)],
                        in_max=vits_sb[:, KT * s + i : KT * s + i + 1].to_broadcast(
                            [128, 8]
                        ),
                        in_values=scores[:, K * i : K * (i + 1)],
                    )

            nc.sync.dma_start(bidx_d.ap(), bidx)

    nc.compile()
    return nc


def _get_programs():
    if "a" not in _CACHE:
        _CACHE["a"] = _build_phase_a()
    if "b" not in _CACHE:
        _CACHE["b"] = _build_phase_b()
    return _CACHE["a"], _CACHE["b"]


def kernel(feats: np.ndarray, transitions: np.ndarray, lengths: np.ndarray):
    feats = np.asarray(feats)
    transitions = np.asarray(transitions, dtype=np.float32)
    lengths = np.asarray(lengths)
    assert feats.shape == (B, T, K) and transitions.shape == (K, K)

    seq = np.ascontiguousarray(feats[SEQ], dtype=np.float32)  # [T, K]
    length = int(lengths[SEQ])

    # host-side layout prep (cheap)
    # T4T[j, 128*(4*i+q) + m] = transitions[128*i + m, 128*q + j]
    t4t = np.ascontiguousarray(
        transitions.reshape(KT, 128, KT, 128)
        .transpose(3, 0, 2, 1)
        .reshape(128, KT * KT * 128)
    )
    # row layout for phase B's DVE-add tile: t4[p, 512i+n] = transitions[128i+p, n]
    t4 = np.ascontiguousarray(
        transitions.reshape(KT, 128, K).transpose(1, 0, 2).reshape(128, KT * K)
    )
    f4 = np.ascontiguousarray(
        seq.reshape(T, KT, 128).transpose(2, 0, 1).reshape(128, T * KT)
    )
    s0 = np.full(K, NEG_INF, dtype=np.float32)
    s0[START_TAG] = 0.0
    vf0 = np.ascontiguousarray(s0.reshape(KT, 128).T)
    ident = np.eye(128, dtype=np.float32)

    nc_a, nc_b = _get_programs()

    if PROFILE and "model_a" not in LAST_EXEC_NS:
        # No NTFF profiling under axon in this deployment; report the
        # instruction-cost-model timeline (same model the Tile scheduler uses).
        from concourse.timeline_sim import TimelineSim

        LAST_EXEC_NS["model_a"] = int(TimelineSim(nc_a).simulate())
        LAST_EXEC_NS["model_b"] = int(TimelineSim(nc_b).simulate())

    res_a = run_bass_kernel_spmd(
        nc_a,
        [{"t4t": t4t, "feat4": f4, "vf0": vf0, "ident": ident}],
        core_ids=[0],
    )
    vfh = res_a.results[0]["vf_hist"]  # [128, 4*(T+1)]
    vith = res_a.results[0]["vit_hist"]  # [128, 4*T]

    in_maps = []
    for c in range(N_CORES):
        in_maps.append(
            {
                "t4t": t4t,
                "t4": t4,
                "vf_slice": np.ascontiguousarray(vfh[:, KT * S * c : KT * S * (c + 1)]),
                "vit_slice": np.ascontiguousarray(
                    vith[:, KT * S * c : KT * S * (c + 1)]
                ),
                "ident": ident,
            }
        )
    res_b = run_bass_kernel_spmd(nc_b, in_maps, core_ids=list(range(N_CORES)))

    # bptrs[t, n] = first argmax over prev
    bptrs = np.empty((T, K), dtype=np.int64)
    for c in range(N_CORES):
        bidx = res_b.results[c]["bidx"]  # [128, 32*S]
        view = bidx.reshape(128, S, KT, 8)[:, :, :, 0]  # [128, S, KT]
        bptrs[S * c : S * (c + 1)] = view.transpose(1, 2, 0).reshape(S, K)

    # s_L (state after `length` steps; the reference freezes beyond that)
    s_l = vfh[:, KT * length : KT * (length + 1)].T.reshape(K)
    terminal = s_l + transitions[STOP_TAG]
    best = int(np.argmax(terminal))
    score = np.float32(terminal[best])

    int_dtype = lengths.dtype if lengths.dtype in (np.int32, np.int64) else np.int64
    path = np.full(T, -1, dtype=int_dtype)
    tag = best
    for t in range(T - 1, -1, -1):
        if t < length:
            path[t] = tag
            tag = int(bptrs[t, tag])
    return score, path
